# revision 24
# baseline (speedup 1.0000x reference)
"""Self-contained Trainium2 Bass kernel for nn_Model_16801912062040 (dense_cnn).

Collective-free sharding: batch-parallel, 2 samples per core across 8 cores.
The only cross-sample coupling is s_o = sum_n alpha[n,o]^2 per layer, where
alpha comes from pooled stats pp[n,c,t] = mean_j x2[n,c,t,j] over all 16
samples.  Instead of AllGathering pp (which serializes every core behind the
slowest-launching one), every core computes pp for ALL samples redundantly:

- Layer 0: pp0 (and the 4 edge columns of x2_0) are linear in the raw input x.
  The torch-style scrambling reshape (B,L,D)->(B*D,1,L) makes the composite
  token-conv+patch-conv filter d-dependent, but it stays shift-invariant in
  nt, giving per-variant filters H[d, (c,tau)] with tau in 0..13.  Computed
  as 15 chunked matmuls over host-built input windows xw[(tau,c), (n,nt)],
  with the pos-embed + bias contribution folded into the last chunk through
  16 indicator rows.
- Layer 1: pp1 = pp0 + rowsum(z_0).  rowsum(z_0) collapses algebraically:
  full-conv row sums need only Y0 row sums (yrs), and yrs needs only x2_0
  row sums (= 24*pp0) and its 4 edge columns.  15 + 3 more matmuls.

Every core then runs fully independently: no collectives, no cross-core
skew coupling.
"""
import math
import numpy as np

import concourse.bass as bass
import concourse.tile as tile
from concourse import bacc, mybir
from concourse.bass_utils import run_bass_kernel_spmd

N_CORES = 8
BPC = 2            # samples per core
B, L, CIN, D = 16, 192, 21, 128
P, S, NT, PRED, COUT = 24, 12, 16, 96, 21
LAYERS = 2
BN_EPS = 1e-5
F32 = mybir.dt.float32
BF16 = mybir.dt.bfloat16

NTAU = 14                      # composite filter support in x
CH = [(0, 5), (5, 10), (10, 14)]   # tau chunks -> rows 105, 105, 84(+16)

_CACHE = {}
LAST_RESULT = None


def _pos_embed():
    pos = np.arange(L, dtype=np.float32)[:, None]
    div = np.exp(np.arange(0, D, 2, dtype=np.float32) * (-math.log(10000.0) / D))
    pe = np.zeros((L, D), np.float32)
    pe[:, 0::2] = np.sin(pos * div)
    pe[:, 1::2] = np.cos(pos * div)
    return pe


def _build_variant(token_w, patch_w, patch_b, pe, p_list):
    """Composite filter for sum_{p in p_list} xe2[b,nt,p,d]:
    H[d,c,tau], posc2[d,nt] (pos-embed + patch bias folded)."""
    dd = np.arange(D)
    H2 = np.zeros((D * NTAU, CIN), np.float32)
    posc = np.zeros((D, NT), np.float32)
    bias = np.zeros((D,), np.float32)
    nts = 12 * np.arange(NT)
    for p_ in p_list:
        g, h = p_ // 3, p_ % 3
        idx = 128 * h + dd                     # (D,)
        ntp, pp_ = idx // 24, idx % 24
        bias += patch_b[pp_]
        for k in range(P):
            off = np.minimum(12 * ntp + k, 191)
            flat = 192 * g + off
            lam, d2 = flat // 128, flat % 128  # (D,)
            w = patch_w[pp_, 0, k]             # (D,)
            for kk in range(3):
                np.add.at(H2, dd * NTAU + lam + kk,
                          w[:, None] * token_w[d2, :, kk])
            posc += w[:, None] * pe[nts[None, :] + lam[:, None], d2[:, None]]
    return H2.reshape(D, NTAU, CIN), posc + bias[:, None]


def _prep_consts(token_w, patch_w, patch_b, Wi, pconv_w, pconv_b, bn_g, bn_b,
                 aconv_w, fc1_w, fc1_b, fc2_w, fc2_b):
    import ml_dtypes
    c = {}
    pe = _pos_embed()
    # token conv lhsT with (k,c) stacked in contraction; rows 63:111 carry the
    # pos-embed as a rank-48 factorization (pe has exact rank 48), so the
    # matmul emits emb+pos directly
    U, S, Vt = np.linalg.svd(pe.astype(np.float64), full_matrices=False)
    R48 = 48
    sqS = np.sqrt(S[:R48])
    c["peU"] = (U[:, :R48] * sqS[None, :]).astype(np.float32)      # [L, 48]
    def tok3(tw, vt):
        t = np.zeros((128, D), np.float32)
        for k in range(3):
            t[k * CIN:(k + 1) * CIN, :] = tw[:, :, k].T
        t[63:63 + R48, :] = vt[:R48] * sqS[:, None]
        return t.astype(ml_dtypes.bfloat16)
    c["tok3A"] = tok3(token_w, Vt)
    c["tok3R"] = tok3(np.roll(token_w, -64, 0), np.roll(Vt, -64, axis=1))
    # patch conv as dense banded matmul over m (xe column), j = nt*24 + p
    W2p = np.zeros((205, 384), np.float32)
    for nt in range(NT):
        for p_ in range(P):
            j = nt * 24 + p_
            for k in range(P):
                m = 12 * nt + k
                W2p[min(m, 191), j] += patch_w[p_, 0, k]   # replicate-pad fold
            W2p[204, j] = patch_b[p_]
    c["w2pa"] = np.ascontiguousarray(W2p[0:128]).astype(ml_dtypes.bfloat16)   # [128,384]
    c["w2pb"] = np.ascontiguousarray(np.concatenate([W2p[128:192], W2p[204:205]], 0)).astype(ml_dtypes.bfloat16)
    A = Wi.transpose(0, 3, 4, 2, 1).reshape(LAYERS, 9, D, D)                  # [l,tap,i,o]
    c["wiT"] = np.ascontiguousarray(A.transpose(2, 0, 1, 3).reshape(D, LAYERS * 9 * D)).astype(ml_dtypes.bfloat16)
    Wf = Wi[:, :, :, ::-1, ::-1]
    Bt = Wf.transpose(0, 3, 4, 1, 2).reshape(LAYERS, 9, D, D)                 # [l,tap,o,i]
    c["w2T"] = np.ascontiguousarray(Bt.transpose(2, 0, 1, 3).reshape(D, LAYERS * 9 * D)).astype(ml_dtypes.bfloat16)
    Ct = pconv_w.transpose(0, 3, 2, 1) / 24.0                                 # [l,k,c,o]
    c["pcvT"] = np.ascontiguousarray(Ct.transpose(2, 0, 1, 3).reshape(D, LAYERS * 3 * D)).astype(ml_dtypes.bfloat16)
    c["awT"] = np.ascontiguousarray((aconv_w.transpose(2, 0, 1) / 16.0).reshape(D, LAYERS * D))
    c["bns"] = np.ascontiguousarray((bn_g / np.sqrt(1.0 + BN_EPS)).T)         # [128,2]
    c["bnb"] = np.ascontiguousarray((pconv_b * (bn_g / np.sqrt(1.0 + BN_EPS)) + bn_b).T)
    c["fc1e"] = np.ascontiguousarray(fc1_w.T.reshape(D, 3, PRED).reshape(D, 3 * PRED)).astype(ml_dtypes.bfloat16)
    c["fc1b"] = np.ascontiguousarray(fc1_b[None, :]).astype(ml_dtypes.bfloat16)
    c["fc2T"] = np.ascontiguousarray(fc2_w.T).astype(ml_dtypes.bfloat16)      # [128,21]
    c["fc2b"] = np.ascontiguousarray(fc2_b[None, :]).astype(ml_dtypes.bfloat16)

    # ---- redundant all-sample stats path ----
    # 5 variants: pp-sum (all p), edge cols j=0,1,22,23 <-> p=0,2,21,23
    variants = [list(range(P)), [0], [2], [21], [23]]
    Hs, poscs = [], []
    for plist in variants:
        Hv, pcv = _build_variant(token_w, patch_w, patch_b, pe, plist)
        Hs.append(Hv)          # [D, NTAU, CIN]
        poscs.append(pcv)      # [D, NT]
    # chunk q lhsT: rows (tau in chunk, c) -> [rows, 5*128]
    for q, (t0, t1) in enumerate(CH):
        rows = (t1 - t0) * CIN
        blk = np.zeros((rows + (16 if q == 2 else 0), 5 * D), np.float32)
        for v in range(5):
            hv = Hs[v][:, t0:t1, :].reshape(D, rows)      # [D, rows]
            blk[:rows, v * D:(v + 1) * D] = hv.T
            if q == 2:
                blk[rows:rows + 16, v * D:(v + 1) * D] = poscs[v].T  # [16, D]
        blk = np.pad(blk, ((0, 128 - blk.shape[0]), (0, 0)))
        c[f"hall{q}"] = np.ascontiguousarray(blk).astype(ml_dtypes.bfloat16)
    # yrs weights: terms [rowsum(x2)=pp0sum, -e0, -e1, -e22, -e23] x 3 taps
    W0 = Wi[0]                                            # (o, i, 3, 3)
    Wrow = W0.sum(-1)                                     # (o,i,dp)
    A0 = W0[:, :, :, 1] + W0[:, :, :, 2]
    A1 = W0[:, :, :, 2]
    A22 = W0[:, :, :, 0]
    A23 = W0[:, :, :, 0] + W0[:, :, :, 1]
    terms = [Wrow, -A0, -A1, -A22, -A23]
    wy = np.zeros((D, 15 * D), np.float32)
    for dp in range(3):
        for t_, T in enumerate(terms):
            wy[:, (dp * 5 + t_) * D:(dp * 5 + t_ + 1) * D] = T[:, :, dp].T  # [i,o]
    c["wy"] = np.ascontiguousarray(wy).astype(ml_dtypes.bfloat16)
    Wrow2 = W0[:, :, ::-1, :].sum(-1)                     # flipped rows: (o,i,dp)
    wz = np.zeros((D, 3 * D), np.float32)
    for dp in range(3):
        wz[:, dp * D:(dp + 1) * D] = Wrow2[:, :, dp]      # [o,i] contract over o
    c["wz"] = np.ascontiguousarray(wz).astype(ml_dtypes.bfloat16)
    return c


def _build_xw(x):
    """Input windows for the composite filters: xw_q[(tau,c), (n,nt)]."""
    xt = np.pad(x.transpose(0, 2, 1), ((0, 0), (0, 0), (1, 1)),
                mode="wrap").astype(np.float32)           # (B, CIN, 194)
    V = np.stack([xt[:, :, t:t + 181:12] for t in range(NTAU)], 0)  # [14,B,CIN,16]
    V = V.transpose(0, 2, 1, 3).reshape(NTAU * CIN, B * NT)         # rows (tau,c)
    out = []
    for q, (t0, t1) in enumerate(CH):
        blk = V[t0 * CIN:t1 * CIN]
        if q == 2:
            # indicator rows; cols are n-major: col = n*16+t
            ntind = np.zeros((NT, B * NT), np.float32)
            ntind[np.arange(NT)[:, None], np.arange(B)[None, :] * NT + np.arange(NT)[:, None]] = 1.0
            blk = np.concatenate([blk, ntind], 0)
        import ml_dtypes
        blk = np.pad(blk, ((0, 128 - blk.shape[0]), (0, 0)))
        out.append(np.ascontiguousarray(blk).astype(ml_dtypes.bfloat16))
    return out, xt


def _build():
    nc = bacc.Bacc("TRN2", target_bir_lowering=False, debug=False, num_devices=N_CORES)

    def param(name, shape, dt=F32):
        return nc.declare_dram_parameter(name, list(shape), dt, isOutput=False)

    xt3p = param("xt3p", (128, BPC, L), BF16)
    xw = [param(f"xw{q}", (128, B * NT), BF16) for q in range(3)]
    hall = [param(f"hall{q}", (128, 5 * D), BF16) for q in range(3)]
    tok3A = param("tok3A", (128, D), BF16); tok3R = param("tok3R", (128, D), BF16)
    w2pa = param("w2pa", (D, 384), BF16); w2pb = param("w2pb", (65, 384), BF16)
    wiT = param("wiT", (D, LAYERS * 9 * D), BF16)
    w2T = param("w2T", (D, LAYERS * 9 * D), BF16)
    pcvT = param("pcvT", (D, LAYERS * 3 * D), BF16); awT = param("awT", (D, LAYERS * D))
    bns = param("bns", (D, LAYERS)); bnb = param("bnb", (D, LAYERS))
    wy = param("wy", (D, 15 * D), BF16); wz = param("wz", (D, 3 * D), BF16)
    fc1e = param("fc1e", (D, 3 * PRED), BF16); fc1b = param("fc1b", (1, PRED), BF16)
    fc2T = param("fc2T", (D, COUT), BF16); fc2b = param("fc2b", (1, COUT), BF16)
    out = nc.declare_dram_parameter("out", [BPC, PRED, COUT], F32, isOutput=True)

    RELU = mybir.ActivationFunctionType.Relu
    SQUARE = mybir.ActivationFunctionType.Square
    ADD = mybir.AluOpType.add
    AX = mybir.AxisListType.X

    with tile.TileContext(nc) as tc:
        with tc.tile_pool(name="w", bufs=1) as wp, \
             tc.tile_pool(name="act", bufs=2) as ap, \
             tc.tile_pool(name="x2p", bufs=6) as xp, \
             tc.tile_pool(name="psv", bufs=1, space="PSUM") as pv, \
             tc.tile_pool(name="ps", bufs=1, space="PSUM") as pp:

            def wload(eng, handle, shape, tag, dt=F32):
                t = wp.tile(list(shape), dt, tag=tag)
                eng.dma_start(out=t[:], in_=handle[tuple(slice(None) for _ in shape)])
                return t

            # --- critical-path DMAs first, spread across engine queues ---
            # (scalar's queue is blocked by ACT_TABLE_LOAD until ~11.3us, so
            #  the stats-path inputs go on sync/gpsimd)
            xw_sb = [wload(nc.sync, xw[q], xw[q].shape, f"xw{q}", BF16) for q in range(3)]
            wy_sb = wload(nc.sync, wy, (D, 15 * D), "wy", BF16)
            hall_sb = [wload(nc.gpsimd, hall[q], hall[q].shape, f"hall{q}", BF16)
                       for q in range(3)]
            xt3_sb = ap.tile([128, BPC, L], BF16, tag="xt", bufs=1)
            nc.sync.dma_start(out=xt3_sb[:], in_=xt3p[:, :, :])
            tokA_sb = wload(nc.sync, tok3A, (128, D), "tok3A", BF16)
            awT_sb = wload(nc.gpsimd, awT, (D, LAYERS * D), "awT")
            bns_sb = wload(nc.gpsimd, bns, (D, LAYERS), "bns")
            bnb_sb = wload(nc.gpsimd, bnb, (D, LAYERS), "bnb")
            wz_sb = wload(nc.gpsimd, wz, (D, 3 * D), "wz", BF16)
            pcvT_sb = wload(nc.scalar, pcvT, (D, LAYERS * 3 * D), "pcvT", BF16)
            tokR_sb = wload(nc.scalar, tok3R, (128, D), "tok3R", BF16)
            w2pa_sb = wload(nc.gpsimd, w2pa, (D, 384), "w2pa", BF16)
            w2pb_sb = wload(nc.gpsimd, w2pb, (65, 384), "w2pb", BF16)
            wiT_sb = wload(nc.sync, wiT, (D, LAYERS * 9 * D), "wiT", BF16)
            w2T_sb = wload(nc.sync, w2T, (D, LAYERS * 9 * D), "w2T", BF16)
            fc1e_sb = wload(nc.scalar, fc1e, (D, 3 * PRED), "fc1e", BF16)
            fc1b_sb = wload(nc.scalar, fc1b, (1, PRED), "fc1b", BF16)
            fc2T_sb = wload(nc.scalar, fc2T, (D, COUT), "fc2T", BF16)
            fc2b_sb = wload(nc.scalar, fc2b, (1, COUT), "fc2b", BF16)
            ones_sb = wp.tile([1, D], BF16, tag="ones")
            nc.vector.memset(ones_sb[:], 1.0)

            # ---------------- all-sample stats: pp0 + 4 edge cols ----------------
            # psv tiles: v0,v1 | v2,v3 | v4
            pv01 = pv.tile([D, 2, B * NT], F32, tag="pv01")
            pv23 = pv.tile([D, 2, B * NT], F32, tag="pv23")
            pv4 = pv.tile([D, 1, B * NT], F32, tag="pv4")
            vloc = [(pv01, 0), (pv01, 1), (pv23, 0), (pv23, 1), (pv4, 0)]
            for v in range(5):
                tl, sl = vloc[v]
                for q in range(3):
                    nc.tensor.matmul(tl[:, sl, :], lhsT=hall_sb[q][:, v * D:(v + 1) * D],
                                     rhs=xw_sb[q][:], start=(q == 0), stop=(q == 2))
            pp0_sb = ap.tile([D, B, NT], BF16, tag="pp0")
            nc.scalar.copy(out=pp0_sb[:], in_=pv01[:, 0, :].rearrange("d (n t) -> d n t", t=NT))
            e_sbs = []
            for v in range(1, 5):
                tl, sl = vloc[v]
                e_t = ap.tile([D, B, NT], BF16, tag=f"ev{v}")
                if v in (1, 3):
                    nc.vector.tensor_copy(out=e_t[:], in_=tl[:, sl, :].rearrange("d (n t) -> d n t", t=NT))
                else:
                    nc.scalar.copy(out=e_t[:], in_=tl[:, sl, :].rearrange("d (n t) -> d n t", t=NT))
                e_sbs.append(e_t)

            def alpha_conv_mm(l, ppv_sb):
                ppc_ps = pp.tile([D, B, NT], F32, tag="ps", bufs=4)
                for k in range(3):
                    o = (l * 3 + k) * D
                    if k == 0:
                        nc.tensor.matmul(ppc_ps[:, :, 1:NT], lhsT=pcvT_sb[:, o:o + D],
                                         rhs=ppv_sb[:, :, 0:NT - 1], start=True, stop=False)
                    elif k == 1:
                        nc.tensor.matmul(ppc_ps[:], lhsT=pcvT_sb[:, o:o + D],
                                         rhs=ppv_sb[:], start=False, stop=False)
                    else:
                        nc.tensor.matmul(ppc_ps[:, :, 0:NT - 1], lhsT=pcvT_sb[:, o:o + D],
                                         rhs=ppv_sb[:, :, 1:NT], start=False, stop=True)
                return ppc_ps

            def alpha_pool(l, ppc_ps):
                ppc_sb = ap.tile([D, B, NT], F32, tag="ppc_sb")
                nc.scalar.activation(out=ppc_sb[:], in_=ppc_ps[:], func=RELU,
                                     bias=bnb_sb[:, l:l + 1], scale=bns_sb[:, l:l + 1])
                pooled = ap.tile([D, B], F32, tag="pooled")
                nc.vector.tensor_reduce(out=pooled[:], in_=ppc_sb[:], axis=AX, op=ADD)
                return pooled

            def alpha_conv(l, ppv_sb):
                return alpha_pool(l, alpha_conv_mm(l, ppv_sb))

            def alpha_sq(l, pooled):
                al_ps = pp.tile([D, B], F32, tag="ps", bufs=4)
                nc.tensor.matmul(al_ps[:], lhsT=awT_sb[:, l * D:(l + 1) * D],
                                 rhs=pooled[:], start=True, stop=True)
                asq = ap.tile([D, B], F32, tag="asq")
                s_t = ap.tile([D, 1], F32, tag=f"s{l}", bufs=1)
                nc.scalar.activation(out=asq[:], in_=al_ps[:], func=SQUARE,
                                     bias=1.0, scale=1.0, accum_out=s_t[:])
                return s_t

            pooled0 = alpha_conv(0, pp0_sb)

            # ---------------- yrs (layer-1 stats precompute, s-independent) ----
            yrs_ps = pp.tile([D, B, 14], F32, tag="yrs", bufs=1)
            rhs5 = [pp0_sb] + e_sbs
            mi = 0
            for dp in range(3):
                for t_ in range(5):
                    nc.tensor.matmul(yrs_ps[:], lhsT=wy_sb[:, mi * D:(mi + 1) * D],
                                     rhs=rhs5[t_][:, :, dp:dp + 14],
                                     start=(mi == 0), stop=(mi == 14))
                    mi += 1
            s0_sb = alpha_sq(0, pooled0)

            # s1 chain start: ysp = s0 * yrs (vector picks this up the moment
            # s0 lands, before the embedding evacuations queue behind it)
            ysp = ap.tile([D, B, 18], BF16, tag="ysp")
            nc.gpsimd.memset(ysp[:], 0.0)
            nc.vector.tensor_scalar_mul(out=ysp[:, :, 2:16], in0=yrs_ps[:],
                                        scalar1=s0_sb[:])

            # ---------------- own-sample embedding (emb+pos in one matmul) ----
            emb2 = []
            for vi, tok_sb in enumerate((tokA_sb, tokR_sb)):
                e_ps = pp.tile([D, BPC, L], F32, tag="ps", bufs=4)
                nc.tensor.matmul(e_ps[:], lhsT=tok_sb[:], rhs=xt3_sb[:],
                                 start=True, stop=True)
                e_sb = ap.tile([D, BPC, L], BF16, tag=f"emb_sb{vi}", bufs=1)
                nc.vector.tensor_copy(out=e_sb[:], in_=e_ps[:])
                emb2.append(e_sb)

            # ---------------- Zrs, pp1 ----
            zrs_ps = pp.tile([D, B, NT], F32, tag="ps", bufs=4)
            for dp in range(3):
                nc.tensor.matmul(zrs_ps[:], lhsT=wz_sb[:, dp * D:(dp + 1) * D],
                                 rhs=ysp[:, :, dp:dp + 16],
                                 start=(dp == 0), stop=(dp == 2))
            pp1_sb = ap.tile([D, B, NT], BF16, tag="pp1")
            nc.vector.tensor_add(out=pp1_sb[:], in0=zrs_ps[:], in1=pp0_sb[:])

            # ---------------- patch conv, interleaved with the s1 alpha chain --
            x2 = []
            pcs2 = []
            pooled1 = None
            for n in range(BPC):
                eT = emb2[0][:, n, :].rearrange("p (s c) -> p s c", c=3)
                eR = emb2[1][:, n, :].rearrange("p (s c) -> p s c", c=3)
                xeA = ap.tile([D, 64, 2], BF16, tag="xeA")
                xeB = ap.tile([65, 64, 2], BF16, tag="xeB")
                nc.gpsimd.tensor_copy(out=xeA[:, :, 0], in_=eT[:, :, 0])
                nc.gpsimd.tensor_copy(out=xeA[0:64, :, 1], in_=eR[0:64, :, 1])
                nc.gpsimd.tensor_copy(out=xeA[64:128, :, 1], in_=eR[64:128, :, 2])
                nc.scalar.copy(out=xeB[0:64, :, 0], in_=eT[0:64, :, 1])
                nc.scalar.copy(out=xeB[0:64, :, 1], in_=eR[0:64, :, 2])
                nc.vector.memset(xeB[64:65, :, :], 1.0)
                pcs = []
                for e in range(3):
                    pc_ps = pp.tile([D, NT, 8], F32, tag="ps", bufs=4)
                    nc.tensor.matmul(pc_ps[:], lhsT=w2pa_sb[:, 128 * e:128 * (e + 1)],
                                     rhs=xeA[:], start=True, stop=False)
                    nc.tensor.matmul(pc_ps[:], lhsT=w2pb_sb[:, 128 * e:128 * (e + 1)],
                                     rhs=xeB[:], start=False, stop=True)
                    pcs.append(pc_ps)
                pcs2.append(pcs)
                if n == 0:
                    ppc1_ps = alpha_conv_mm(1, pp1_sb)
            s1_sb = alpha_sq(1, alpha_pool(1, ppc1_ps))
            s_list = [s0_sb, s1_sb]

            for n in range(BPC):
                # x2 col for i2=3q+e is 12*(i2%2)+i2//2; even-i2 col c pairs with
                # odd-i2 col c+13 at the next q -> one strided 2-col copy
                x2n = xp.tile([D, NT, 24], BF16, tag="x2")
                ci = 0
                for e in range(3):
                    q = 0
                    while q < 8:
                        i2 = 3 * q + e
                        col = 12 * (i2 % 2) + i2 // 2
                        eng = [nc.vector.tensor_copy,
                               lambda out, in_: nc.scalar.copy(out=out, in_=in_)][ci % 2]
                        if i2 % 2 == 0 and col <= 10 and q + 1 < 8:
                            eng(out=x2n[:, :, col:col + 14:13],
                                in_=pcs2[n][e][:, :, q:q + 2])
                            q += 2
                        else:
                            eng(out=x2n[:, :, col], in_=pcs2[n][e][:, :, q])
                            q += 1
                        ci += 1
                x2.append(x2n)

            # ---------------- TimesBlocks (own samples) ----------------
            for l in range(LAYERS):
                s_cur = s_list[l]
                y0ps = []
                for n in range(BPC):
                    y0_ps = pp.tile([D, 14, 22], F32, tag="ps", bufs=4)
                    for dp in range(3):
                        for dq in range(3):
                            tap = 3 * dp + dq
                            o = (l * 9 + tap) * D
                            nc.tensor.matmul(y0_ps[:], lhsT=wiT_sb[:, o:o + D],
                                             rhs=x2[n][:, dp:dp + 14, dq:dq + 22],
                                             start=(tap == 0), stop=(tap == 8))
                    y0ps.append(y0_ps)

                y0pads = []
                for n in range(BPC):
                    y0p = ap.tile([D, 18, 26], BF16, tag="y0p", bufs=2)
                    nc.gpsimd.memset(y0p[:], 0.0)
                    y0pads.append(y0p)
                for n in range(BPC):
                    y0p = y0pads[n]
                    nc.vector.tensor_scalar_mul(out=y0p[:, 2:16, 2:24],
                                                in0=y0ps[n][:], scalar1=s_cur[:])
                    z_ps = pp.tile([D, NT, 24], F32, tag="ps", bufs=4)
                    for dp in range(3):
                        for dq in range(3):
                            tap = 3 * dp + dq
                            o = (l * 9 + tap) * D
                            nc.tensor.matmul(z_ps[:], lhsT=w2T_sb[:, o:o + D],
                                             rhs=y0p[:, dp:dp + 16, dq:dq + 24],
                                             start=(tap == 0), stop=(tap == 8))
                    x2n = xp.tile([D, NT, 24], BF16, tag="x2")
                    nc.vector.tensor_add(out=x2n[:], in0=z_ps[:], in1=x2[n][:])
                    x2[n] = x2n

            # ---------------- heads ----------------
            y1_sbs = []
            for n in range(BPC):
                x2f = x2[n][:].rearrange("p a b -> p (a b)")
                y1_ps = pp.tile([D, PRED], F32, tag="ps", bufs=4)
                for e in range(3):
                    nc.tensor.matmul(y1_ps[:], lhsT=x2f[:, 128 * e:128 * (e + 1)],
                                     rhs=fc1e_sb[:, PRED * e:PRED * (e + 1)],
                                     start=(e == 0), stop=False)
                nc.tensor.matmul(y1_ps[:], lhsT=ones_sb[:], rhs=fc1b_sb[:],
                                 start=False, stop=True)
                y1_sb = ap.tile([D, PRED], BF16, tag="y1sb", bufs=2)
                eng = nc.scalar.copy if n == 0 else (
                    lambda out, in_: nc.vector.tensor_copy(out=out, in_=in_))
                eng(out=y1_sb[:], in_=y1_ps[:])
                y1_sbs.append(y1_sb)
            o2_sb = ap.tile([PRED, BPC, COUT], F32, tag="o2sb")
            for n in range(BPC):
                o_ps = pp.tile([PRED, COUT], F32, tag="ps", bufs=4)
                nc.tensor.matmul(o_ps[:], lhsT=y1_sbs[n][:], rhs=fc2T_sb[:],
                                 start=True, stop=False)
                nc.tensor.matmul(o_ps[:], lhsT=ones_sb[:, 0:PRED], rhs=fc2b_sb[:],
                                 start=False, stop=True)
                if n == 0:
                    nc.vector.tensor_copy(out=o2_sb[:, n, :], in_=o_ps[:])
                else:
                    nc.scalar.copy(out=o2_sb[:, n, :], in_=o_ps[:])
            nc.sync.dma_start(out=out[:, :, :].rearrange("n p c -> p n c"),
                              in_=o2_sb[:])

    nc.finalize()
    return nc


def kernel(**inputs):
    global LAST_RESULT
    inputs = {k: np.ascontiguousarray(np.asarray(v, np.float32)) for k, v in inputs.items()}
    if "nc" not in _CACHE:
        _CACHE["nc"] = _build()
    nc = _CACHE["nc"]
    c = _prep_consts(
        inputs["token_w"], inputs["patch_w"], inputs["patch_b"], inputs["Wi"],
        inputs["pconv_w"], inputs["pconv_b"], inputs["bn_g"], inputs["bn_b"],
        inputs["aconv_w"], inputs["fc1_w"], inputs["fc1_b"], inputs["fc2_w"],
        inputs["fc2_b"])
    xws, xtp_full = _build_xw(inputs["x"])
    for q in range(3):
        c[f"xw{q}"] = xws[q]
    import ml_dtypes
    in_maps = []
    for core in range(N_CORES):
        m = dict(c)
        xt = xtp_full[BPC * core:BPC * (core + 1)]        # (BPC, CIN, 194)
        x3 = np.zeros((128, BPC, L), np.float32)
        for k in range(3):
            x3[k * CIN:(k + 1) * CIN] = xt[:, :, k:k + L].transpose(1, 0, 2)
        x3[63:63 + 48] = c["peU"].T[:, None, :]
        m["xt3p"] = np.ascontiguousarray(x3).astype(ml_dtypes.bfloat16)
        in_maps.append(m)
    import os
    res = run_bass_kernel_spmd(nc, in_maps, core_ids=list(range(N_CORES)),
                               trace=bool(os.environ.get("BASS_TRACE")))
    LAST_RESULT = res
    return np.concatenate([res.results[cid]["out"] for cid in range(N_CORES)], axis=0)


# revision 25
# speedup vs baseline: 1.0107x; 1.0107x over previous
"""Self-contained Trainium2 Bass kernel for nn_Model_16801912062040 (dense_cnn).

Collective-free sharding: batch-parallel, 2 samples per core across 8 cores.
The only cross-sample coupling is s_o = sum_n alpha[n,o]^2 per layer, where
alpha comes from pooled stats pp[n,c,t] = mean_j x2[n,c,t,j] over all 16
samples.  Instead of AllGathering pp (which serializes every core behind the
slowest-launching one), every core computes pp for ALL samples redundantly:

- Layer 0: pp0 (and the 4 edge columns of x2_0) are linear in the raw input x.
  The torch-style scrambling reshape (B,L,D)->(B*D,1,L) makes the composite
  token-conv+patch-conv filter d-dependent, but it stays shift-invariant in
  nt, giving per-variant filters H[d, (c,tau)] with tau in 0..13.  Computed
  as 15 chunked matmuls over host-built input windows xw[(tau,c), (n,nt)],
  with the pos-embed + bias contribution folded into the last chunk through
  16 indicator rows.
- Layer 1: pp1 = pp0 + rowsum(z_0).  rowsum(z_0) collapses algebraically:
  full-conv row sums need only Y0 row sums (yrs), and yrs needs only x2_0
  row sums (= 24*pp0) and its 4 edge columns.  15 + 3 more matmuls.

Every core then runs fully independently: no collectives, no cross-core
skew coupling.
"""
import math
import numpy as np

import concourse.bass as bass
import concourse.tile as tile
from concourse import bacc, mybir
from concourse.bass_utils import run_bass_kernel_spmd

N_CORES = 8
BPC = 2            # samples per core
B, L, CIN, D = 16, 192, 21, 128
P, S, NT, PRED, COUT = 24, 12, 16, 96, 21
LAYERS = 2
BN_EPS = 1e-5
F32 = mybir.dt.float32
BF16 = mybir.dt.bfloat16

NTAU = 14                      # composite filter support in x
CH = [(0, 5), (5, 10), (10, 14)]   # tau chunks -> rows 105, 105, 84(+16)

_CACHE = {}
LAST_RESULT = None


def _pos_embed():
    pos = np.arange(L, dtype=np.float32)[:, None]
    div = np.exp(np.arange(0, D, 2, dtype=np.float32) * (-math.log(10000.0) / D))
    pe = np.zeros((L, D), np.float32)
    pe[:, 0::2] = np.sin(pos * div)
    pe[:, 1::2] = np.cos(pos * div)
    return pe


def _build_variant(token_w, patch_w, patch_b, pe, p_list):
    """Composite filter for sum_{p in p_list} xe2[b,nt,p,d]:
    H[d,c,tau], posc2[d,nt] (pos-embed + patch bias folded)."""
    dd = np.arange(D)
    H2 = np.zeros((D * NTAU, CIN), np.float32)
    posc = np.zeros((D, NT), np.float32)
    bias = np.zeros((D,), np.float32)
    nts = 12 * np.arange(NT)
    for p_ in p_list:
        g, h = p_ // 3, p_ % 3
        idx = 128 * h + dd                     # (D,)
        ntp, pp_ = idx // 24, idx % 24
        bias += patch_b[pp_]
        for k in range(P):
            off = np.minimum(12 * ntp + k, 191)
            flat = 192 * g + off
            lam, d2 = flat // 128, flat % 128  # (D,)
            w = patch_w[pp_, 0, k]             # (D,)
            for kk in range(3):
                np.add.at(H2, dd * NTAU + lam + kk,
                          w[:, None] * token_w[d2, :, kk])
            posc += w[:, None] * pe[nts[None, :] + lam[:, None], d2[:, None]]
    return H2.reshape(D, NTAU, CIN), posc + bias[:, None]


def _prep_consts(token_w, patch_w, patch_b, Wi, pconv_w, pconv_b, bn_g, bn_b,
                 aconv_w, fc1_w, fc1_b, fc2_w, fc2_b):
    import ml_dtypes
    c = {}
    pe = _pos_embed()
    # token conv lhsT with (k,c) stacked in contraction; rows 63:111 carry the
    # pos-embed as a rank-48 factorization (pe has exact rank 48), so the
    # matmul emits emb+pos directly
    U, S, Vt = np.linalg.svd(pe.astype(np.float64), full_matrices=False)
    R48 = 48
    sqS = np.sqrt(S[:R48])
    c["peU"] = (U[:, :R48] * sqS[None, :]).astype(np.float32)      # [L, 48]
    def tok3(tw, vt):
        t = np.zeros((128, D), np.float32)
        for k in range(3):
            t[k * CIN:(k + 1) * CIN, :] = tw[:, :, k].T
        t[63:63 + R48, :] = vt[:R48] * sqS[:, None]
        return t.astype(ml_dtypes.bfloat16)
    c["tok3A"] = tok3(token_w, Vt)
    c["tok3R"] = tok3(np.roll(token_w, -64, 0), np.roll(Vt, -64, axis=1))
    # patch conv as dense banded matmul over m (xe column), j = nt*24 + p
    W2p = np.zeros((205, 384), np.float32)
    for nt in range(NT):
        for p_ in range(P):
            j = nt * 24 + p_
            for k in range(P):
                m = 12 * nt + k
                W2p[min(m, 191), j] += patch_w[p_, 0, k]   # replicate-pad fold
            W2p[204, j] = patch_b[p_]
    c["w2pa"] = np.ascontiguousarray(W2p[0:128]).astype(ml_dtypes.bfloat16)   # [128,384]
    c["w2pb"] = np.ascontiguousarray(np.concatenate([W2p[128:192], W2p[204:205]], 0)).astype(ml_dtypes.bfloat16)
    A = Wi.transpose(0, 3, 4, 2, 1).reshape(LAYERS, 9, D, D)                  # [l,tap,i,o]
    c["wiT"] = np.ascontiguousarray(A.transpose(2, 0, 1, 3).reshape(D, LAYERS * 9 * D)).astype(ml_dtypes.bfloat16)
    Wf = Wi[:, :, :, ::-1, ::-1]
    Bt = Wf.transpose(0, 3, 4, 1, 2).reshape(LAYERS, 9, D, D)                 # [l,tap,o,i]
    c["w2T"] = np.ascontiguousarray(Bt.transpose(2, 0, 1, 3).reshape(D, LAYERS * 9 * D)).astype(ml_dtypes.bfloat16)
    Ct = pconv_w.transpose(0, 3, 2, 1) / 24.0                                 # [l,k,c,o]
    c["pcvT"] = np.ascontiguousarray(Ct.transpose(2, 0, 1, 3).reshape(D, LAYERS * 3 * D)).astype(ml_dtypes.bfloat16)
    c["awT"] = np.ascontiguousarray((aconv_w.transpose(2, 0, 1) / 16.0).reshape(D, LAYERS * D))
    c["bns"] = np.ascontiguousarray((bn_g / np.sqrt(1.0 + BN_EPS)).T)         # [128,2]
    c["bnb"] = np.ascontiguousarray((pconv_b * (bn_g / np.sqrt(1.0 + BN_EPS)) + bn_b).T)
    c["fc1e"] = np.ascontiguousarray(fc1_w.T.reshape(D, 3, PRED).reshape(D, 3 * PRED)).astype(ml_dtypes.bfloat16)
    c["fc1b"] = np.ascontiguousarray(fc1_b[None, :]).astype(ml_dtypes.bfloat16)
    c["fc2T"] = np.ascontiguousarray(fc2_w.T).astype(ml_dtypes.bfloat16)      # [128,21]
    c["fc2b"] = np.ascontiguousarray(fc2_b[None, :]).astype(ml_dtypes.bfloat16)

    # ---- redundant all-sample stats path ----
    # pp0 variant filter (sum over all patch rows)
    variants = [list(range(P)), [0], [2], [21], [23]]
    Hs, poscs = [], []
    for plist in variants:
        Hv, pcv = _build_variant(token_w, patch_w, patch_b, pe, plist)
        Hs.append(Hv)          # [D, NTAU, CIN]
        poscs.append(pcv)      # [D, NT]
    hall = np.zeros((128, 3 * D), np.float32)
    for q, (t0, t1) in enumerate(CH):
        rows = (t1 - t0) * CIN
        hall[:rows, q * D:(q + 1) * D] = Hs[0][:, t0:t1, :].reshape(D, rows).T
        if q == 2:
            hall[rows:rows + 16, q * D:(q + 1) * D] = poscs[0].T
    c["hall"] = np.ascontiguousarray(hall).astype(ml_dtypes.bfloat16)
    # fused yrs filter: yrs[o,n,r] = sum_{tau',c} G2[(tau',c),o]*x[n,c,tau'+12r]
    # + Cst[r,o]  (edge variants and the 5x3 yrs weights folded on host)
    W0 = Wi[0]                                            # (o, i, 3, 3)
    Wrow = W0.sum(-1)
    A0 = W0[:, :, :, 1] + W0[:, :, :, 2]
    A1 = W0[:, :, :, 2]
    A22 = W0[:, :, :, 0]
    A23 = W0[:, :, :, 0] + W0[:, :, :, 1]
    terms = [Wrow, -A0, -A1, -A22, -A23]
    G2 = np.zeros((38 * CIN, D), np.float32)
    Cst = np.zeros((14, D), np.float32)
    for T, Hv, pc2 in zip(terms, Hs, poscs):
        for dp in range(3):
            TT = T[:, :, dp]                              # [o, i']
            blk = np.einsum("itc,oi->tco", Hv, TT).reshape(NTAU * CIN, D)
            r0 = 12 * dp * CIN
            G2[r0:r0 + NTAU * CIN] += blk
            Cst += np.einsum("ir,oi->ro", pc2[:, dp:dp + 14], TT)
    G2a = np.concatenate([G2, Cst], 0)                    # [812, D]
    hy = np.zeros((128, 7 * D), np.float32)
    for ci in range(7):
        hy[:116, ci * D:(ci + 1) * D] = G2a[116 * ci:116 * (ci + 1)]
    c["hy"] = np.ascontiguousarray(hy).astype(ml_dtypes.bfloat16)
    Wrow2 = W0[:, :, ::-1, :].sum(-1)                     # flipped rows: (o,i,dp)
    wz = np.zeros((D, 3 * D), np.float32)
    for dp in range(3):
        wz[:, dp * D:(dp + 1) * D] = Wrow2[:, :, dp]      # [o,i] contract over o
    c["wz"] = np.ascontiguousarray(wz).astype(ml_dtypes.bfloat16)
    return c


def _build_xw(x):
    """Input windows for the composite filters: xw_q[(tau,c), (n,nt)]."""
    xt = np.pad(x.transpose(0, 2, 1), ((0, 0), (0, 0), (1, 1)),
                mode="wrap").astype(np.float32)           # (B, CIN, 194)
    V = np.stack([xt[:, :, t:t + 181:12] for t in range(NTAU)], 0)  # [14,B,CIN,16]
    V = V.transpose(0, 2, 1, 3).reshape(NTAU * CIN, B * NT)         # rows (tau,c)
    import ml_dtypes
    out = []
    for q, (t0, t1) in enumerate(CH):
        blk = V[t0 * CIN:t1 * CIN]
        if q == 2:
            # indicator rows; cols are n-major: col = n*16+t
            ntind = np.zeros((NT, B * NT), np.float32)
            ntind[np.arange(NT)[:, None], np.arange(B)[None, :] * NT + np.arange(NT)[:, None]] = 1.0
            blk = np.concatenate([blk, ntind], 0)
        blk = np.pad(blk, ((0, 128 - blk.shape[0]), (0, 0)))
        out.append(np.ascontiguousarray(blk).astype(ml_dtypes.bfloat16))
    # windows for the fused yrs filter: rows (tau' 0..37, c), cols (n, r 0..13)
    Vq = np.stack([xt[:, :, t:t + 12 * 13 + 1:12][:, :, :14] for t in range(38)], 0)
    Vq = Vq.transpose(0, 2, 1, 3).reshape(38 * CIN, B * 14)         # [798, 224]
    rind = np.zeros((14, B * 14), np.float32)
    rind[np.arange(14)[:, None], np.arange(B)[None, :] * 14 + np.arange(14)[:, None]] = 1.0
    Vq = np.concatenate([Vq, rind], 0)                              # [812, 224]
    xq = np.zeros((128, 7 * B * 14), np.float32)
    for ci in range(7):
        xq[:116, ci * B * 14:(ci + 1) * B * 14] = Vq[116 * ci:116 * (ci + 1)]
    xq = np.ascontiguousarray(xq).astype(ml_dtypes.bfloat16)
    return out, xq, xt


def _build():
    nc = bacc.Bacc("TRN2", target_bir_lowering=False, debug=False, num_devices=N_CORES)

    def param(name, shape, dt=F32):
        return nc.declare_dram_parameter(name, list(shape), dt, isOutput=False)

    xt3p = param("xt3p", (128, BPC, L), BF16)
    xw = [param(f"xw{q}", (128, B * NT), BF16) for q in range(3)]
    hall = param("hall", (128, 3 * D), BF16)
    xq = param("xq", (128, 7 * B * 14), BF16)
    hy = param("hy", (128, 7 * D), BF16)
    tok3A = param("tok3A", (128, D), BF16); tok3R = param("tok3R", (128, D), BF16)
    w2pa = param("w2pa", (D, 384), BF16); w2pb = param("w2pb", (65, 384), BF16)
    wiT = param("wiT", (D, LAYERS * 9 * D), BF16)
    w2T = param("w2T", (D, LAYERS * 9 * D), BF16)
    pcvT = param("pcvT", (D, LAYERS * 3 * D), BF16); awT = param("awT", (D, LAYERS * D))
    bns = param("bns", (D, LAYERS)); bnb = param("bnb", (D, LAYERS))
    wz = param("wz", (D, 3 * D), BF16)
    fc1e = param("fc1e", (D, 3 * PRED), BF16); fc1b = param("fc1b", (1, PRED), BF16)
    fc2T = param("fc2T", (D, COUT), BF16); fc2b = param("fc2b", (1, COUT), BF16)
    out = nc.declare_dram_parameter("out", [BPC, PRED, COUT], F32, isOutput=True)

    RELU = mybir.ActivationFunctionType.Relu
    SQUARE = mybir.ActivationFunctionType.Square
    ADD = mybir.AluOpType.add
    AX = mybir.AxisListType.X

    with tile.TileContext(nc) as tc:
        with tc.tile_pool(name="w", bufs=1) as wp, \
             tc.tile_pool(name="act", bufs=2) as ap, \
             tc.tile_pool(name="x2p", bufs=6) as xp, \
             tc.tile_pool(name="psv", bufs=1, space="PSUM") as pv, \
             tc.tile_pool(name="ps", bufs=1, space="PSUM") as pp:

            def wload(eng, handle, shape, tag, dt=F32):
                t = wp.tile(list(shape), dt, tag=tag)
                eng.dma_start(out=t[:], in_=handle[tuple(slice(None) for _ in shape)])
                return t

            # --- critical-path DMAs first, spread across engine queues ---
            # (scalar's queue is blocked by ACT_TABLE_LOAD until ~11.3us, so
            #  the stats-path inputs go on sync/gpsimd)
            hall_sb = wload(nc.sync, hall, (128, 3 * D), "hall", BF16)
            xw_sb = [wload(nc.sync, xw[q], xw[q].shape, f"xw{q}", BF16) for q in range(3)]
            xq_sb = wload(nc.gpsimd, xq, (128, 7 * B * 14), "xq", BF16)
            hy_sb = wload(nc.gpsimd, hy, (128, 7 * D), "hy", BF16)
            xt3_sb = ap.tile([128, BPC, L], BF16, tag="xt", bufs=1)
            nc.sync.dma_start(out=xt3_sb[:], in_=xt3p[:, :, :])
            tokA_sb = wload(nc.sync, tok3A, (128, D), "tok3A", BF16)
            awT_sb = wload(nc.gpsimd, awT, (D, LAYERS * D), "awT")
            bns_sb = wload(nc.gpsimd, bns, (D, LAYERS), "bns")
            bnb_sb = wload(nc.gpsimd, bnb, (D, LAYERS), "bnb")
            wz_sb = wload(nc.gpsimd, wz, (D, 3 * D), "wz", BF16)
            pcvT_sb = wload(nc.scalar, pcvT, (D, LAYERS * 3 * D), "pcvT", BF16)
            tokR_sb = wload(nc.scalar, tok3R, (128, D), "tok3R", BF16)
            w2pa_sb = wload(nc.gpsimd, w2pa, (D, 384), "w2pa", BF16)
            w2pb_sb = wload(nc.gpsimd, w2pb, (65, 384), "w2pb", BF16)
            wiT_sb = wload(nc.sync, wiT, (D, LAYERS * 9 * D), "wiT", BF16)
            w2T_sb = wload(nc.sync, w2T, (D, LAYERS * 9 * D), "w2T", BF16)
            fc1e_sb = wload(nc.scalar, fc1e, (D, 3 * PRED), "fc1e", BF16)
            fc1b_sb = wload(nc.scalar, fc1b, (1, PRED), "fc1b", BF16)
            fc2T_sb = wload(nc.scalar, fc2T, (D, COUT), "fc2T", BF16)
            fc2b_sb = wload(nc.scalar, fc2b, (1, COUT), "fc2b", BF16)
            ones_sb = wp.tile([1, D], BF16, tag="ones")
            nc.vector.memset(ones_sb[:], 1.0)

            # ---------------- all-sample stats: pp0 ----------------
            pv0 = pv.tile([D, B * NT], F32, tag="pv0")
            for q in range(3):
                nc.tensor.matmul(pv0[:], lhsT=hall_sb[:, q * D:(q + 1) * D],
                                 rhs=xw_sb[q][:], start=(q == 0), stop=(q == 2))
            pp0_sb = ap.tile([D, B, NT], BF16, tag="pp0")
            nc.scalar.copy(out=pp0_sb[:], in_=pv0[:].rearrange("d (n t) -> d n t", t=NT))

            def alpha_conv_mm(l, ppv_sb):
                ppc_ps = pp.tile([D, B, NT], F32, tag="ps", bufs=6)
                for k in range(3):
                    o = (l * 3 + k) * D
                    if k == 0:
                        nc.tensor.matmul(ppc_ps[:, :, 1:NT], lhsT=pcvT_sb[:, o:o + D],
                                         rhs=ppv_sb[:, :, 0:NT - 1], start=True, stop=False)
                    elif k == 1:
                        nc.tensor.matmul(ppc_ps[:], lhsT=pcvT_sb[:, o:o + D],
                                         rhs=ppv_sb[:], start=False, stop=False)
                    else:
                        nc.tensor.matmul(ppc_ps[:, :, 0:NT - 1], lhsT=pcvT_sb[:, o:o + D],
                                         rhs=ppv_sb[:, :, 1:NT], start=False, stop=True)
                return ppc_ps

            def alpha_pool(l, ppc_ps):
                ppc_sb = ap.tile([D, B, NT], F32, tag="ppc_sb")
                nc.scalar.activation(out=ppc_sb[:], in_=ppc_ps[:], func=RELU,
                                     bias=bnb_sb[:, l:l + 1], scale=bns_sb[:, l:l + 1])
                pooled = ap.tile([D, B], F32, tag="pooled")
                nc.vector.tensor_reduce(out=pooled[:], in_=ppc_sb[:], axis=AX, op=ADD)
                return pooled

            def alpha_conv(l, ppv_sb):
                return alpha_pool(l, alpha_conv_mm(l, ppv_sb))

            def alpha_sq(l, pooled):
                al_ps = pp.tile([D, B], F32, tag="ps", bufs=6)
                nc.tensor.matmul(al_ps[:], lhsT=awT_sb[:, l * D:(l + 1) * D],
                                 rhs=pooled[:], start=True, stop=True)
                asq = ap.tile([D, B], F32, tag="asq")
                s_t = ap.tile([D, 1], F32, tag=f"s{l}", bufs=1)
                nc.scalar.activation(out=asq[:], in_=al_ps[:], func=SQUARE,
                                     bias=1.0, scale=1.0, accum_out=s_t[:])
                return s_t

            pooled0 = alpha_conv(0, pp0_sb)

            # ---- yrs via the fused filter (independent of the H matmuls) ----
            yrs_ps = pp.tile([D, B, 14], F32, tag="yrs", bufs=1)
            for ci in range(7):
                nc.tensor.matmul(yrs_ps[:], lhsT=hy_sb[:, ci * D:(ci + 1) * D],
                                 rhs=xq_sb[:, ci * B * 14:(ci + 1) * B * 14],
                                 start=(ci == 0), stop=(ci == 6))
            s0_sb = alpha_sq(0, pooled0)

            # s1 chain start: ysp = s0 * yrs (vector picks this up the moment
            # s0 lands, before the embedding evacuations queue behind it)
            ysp = ap.tile([D, B, 18], BF16, tag="ysp")
            nc.gpsimd.memset(ysp[:], 0.0)
            nc.vector.tensor_scalar_mul(out=ysp[:, :, 2:16], in0=yrs_ps[:],
                                        scalar1=s0_sb[:])

            # ---------------- own-sample embedding (emb+pos in one matmul) ----
            emb2 = []
            for vi, tok_sb in enumerate((tokA_sb, tokR_sb)):
                e_ps = pp.tile([D, BPC, L], F32, tag="ps", bufs=6)
                nc.tensor.matmul(e_ps[:], lhsT=tok_sb[:], rhs=xt3_sb[:],
                                 start=True, stop=True)
                e_sb = ap.tile([D, BPC, L], BF16, tag=f"emb_sb{vi}", bufs=1)
                nc.vector.tensor_copy(out=e_sb[:], in_=e_ps[:])
                emb2.append(e_sb)

            # ---------------- Zrs, pp1 ----
            zrs_ps = pp.tile([D, B, NT], F32, tag="ps", bufs=6)
            for dp in range(3):
                nc.tensor.matmul(zrs_ps[:], lhsT=wz_sb[:, dp * D:(dp + 1) * D],
                                 rhs=ysp[:, :, dp:dp + 16],
                                 start=(dp == 0), stop=(dp == 2))
            pp1_sb = ap.tile([D, B, NT], BF16, tag="pp1")
            nc.vector.tensor_add(out=pp1_sb[:], in0=zrs_ps[:], in1=pp0_sb[:])

            # ---------------- patch conv, interleaved with the s1 alpha chain --
            x2 = []
            pcs2 = []
            pooled1 = None
            for n in range(BPC):
                eT = emb2[0][:, n, :].rearrange("p (s c) -> p s c", c=3)
                eR = emb2[1][:, n, :].rearrange("p (s c) -> p s c", c=3)
                xeA = ap.tile([D, 64, 2], BF16, tag="xeA")
                xeB = ap.tile([65, 64, 2], BF16, tag="xeB")
                nc.gpsimd.tensor_copy(out=xeA[:, :, 0], in_=eT[:, :, 0])
                nc.gpsimd.tensor_copy(out=xeA[0:64, :, 1], in_=eR[0:64, :, 1])
                nc.gpsimd.tensor_copy(out=xeA[64:128, :, 1], in_=eR[64:128, :, 2])
                nc.scalar.copy(out=xeB[0:64, :, 0], in_=eT[0:64, :, 1])
                nc.scalar.copy(out=xeB[0:64, :, 1], in_=eR[0:64, :, 2])
                nc.vector.memset(xeB[64:65, :, :], 1.0)
                pcs = []
                for e in range(3):
                    pc_ps = pp.tile([D, NT, 8], F32, tag="ps", bufs=6)
                    nc.tensor.matmul(pc_ps[:], lhsT=w2pa_sb[:, 128 * e:128 * (e + 1)],
                                     rhs=xeA[:], start=True, stop=False)
                    nc.tensor.matmul(pc_ps[:], lhsT=w2pb_sb[:, 128 * e:128 * (e + 1)],
                                     rhs=xeB[:], start=False, stop=True)
                    pcs.append(pc_ps)
                pcs2.append(pcs)
                if n == 0:
                    ppc1_ps = alpha_conv_mm(1, pp1_sb)
            s1_sb = alpha_sq(1, alpha_pool(1, ppc1_ps))
            s_list = [s0_sb, s1_sb]

            for n in range(BPC):
                # x2 col for i2=3q+e is 12*(i2%2)+i2//2; even-i2 col c pairs with
                # odd-i2 col c+13 at the next q -> one strided 2-col copy
                x2n = xp.tile([D, NT, 24], BF16, tag="x2")
                ci = 0
                for e in range(3):
                    q = 0
                    while q < 8:
                        i2 = 3 * q + e
                        col = 12 * (i2 % 2) + i2 // 2
                        eng = [nc.vector.tensor_copy,
                               lambda out, in_: nc.scalar.copy(out=out, in_=in_)][ci % 2]
                        if i2 % 2 == 0 and col <= 10 and q + 1 < 8:
                            eng(out=x2n[:, :, col:col + 14:13],
                                in_=pcs2[n][e][:, :, q:q + 2])
                            q += 2
                        else:
                            eng(out=x2n[:, :, col], in_=pcs2[n][e][:, :, q])
                            q += 1
                        ci += 1
                x2.append(x2n)

            # ---------------- TimesBlocks (own samples) ----------------
            for l in range(LAYERS):
                s_cur = s_list[l]
                y0ps = []
                for n in range(BPC):
                    y0_ps = pp.tile([D, 14, 22], F32, tag="ps", bufs=6)
                    for dp in range(3):
                        for dq in range(3):
                            tap = 3 * dp + dq
                            o = (l * 9 + tap) * D
                            nc.tensor.matmul(y0_ps[:], lhsT=wiT_sb[:, o:o + D],
                                             rhs=x2[n][:, dp:dp + 14, dq:dq + 22],
                                             start=(tap == 0), stop=(tap == 8))
                    y0ps.append(y0_ps)

                y0pads = []
                for n in range(BPC):
                    y0p = ap.tile([D, 18, 26], BF16, tag="y0p", bufs=2)
                    nc.gpsimd.memset(y0p[:], 0.0)
                    y0pads.append(y0p)
                for n in range(BPC):
                    y0p = y0pads[n]
                    nc.vector.tensor_scalar_mul(out=y0p[:, 2:16, 2:24],
                                                in0=y0ps[n][:], scalar1=s_cur[:])
                    z_ps = pp.tile([D, NT, 24], F32, tag="ps", bufs=6)
                    for dp in range(3):
                        for dq in range(3):
                            tap = 3 * dp + dq
                            o = (l * 9 + tap) * D
                            nc.tensor.matmul(z_ps[:], lhsT=w2T_sb[:, o:o + D],
                                             rhs=y0p[:, dp:dp + 16, dq:dq + 24],
                                             start=(tap == 0), stop=(tap == 8))
                    x2n = xp.tile([D, NT, 24], BF16, tag="x2")
                    nc.vector.tensor_add(out=x2n[:], in0=z_ps[:], in1=x2[n][:])
                    x2[n] = x2n

            # ---------------- heads ----------------
            y1_sbs = []
            for n in range(BPC):
                x2f = x2[n][:].rearrange("p a b -> p (a b)")
                y1_ps = pp.tile([D, PRED], F32, tag="ps", bufs=6)
                for e in range(3):
                    nc.tensor.matmul(y1_ps[:], lhsT=x2f[:, 128 * e:128 * (e + 1)],
                                     rhs=fc1e_sb[:, PRED * e:PRED * (e + 1)],
                                     start=(e == 0), stop=False)
                nc.tensor.matmul(y1_ps[:], lhsT=ones_sb[:], rhs=fc1b_sb[:],
                                 start=False, stop=True)
                y1_sb = ap.tile([D, PRED], BF16, tag="y1sb", bufs=2)
                eng = nc.scalar.copy if n == 0 else (
                    lambda out, in_: nc.vector.tensor_copy(out=out, in_=in_))
                eng(out=y1_sb[:], in_=y1_ps[:])
                y1_sbs.append(y1_sb)
            o2_sb = ap.tile([PRED, BPC, COUT], F32, tag="o2sb")
            for n in range(BPC):
                o_ps = pp.tile([PRED, COUT], F32, tag="ps", bufs=6)
                nc.tensor.matmul(o_ps[:], lhsT=y1_sbs[n][:], rhs=fc2T_sb[:],
                                 start=True, stop=False)
                nc.tensor.matmul(o_ps[:], lhsT=ones_sb[:, 0:PRED], rhs=fc2b_sb[:],
                                 start=False, stop=True)
                if n == 0:
                    nc.vector.tensor_copy(out=o2_sb[:, n, :], in_=o_ps[:])
                else:
                    nc.scalar.copy(out=o2_sb[:, n, :], in_=o_ps[:])
            nc.sync.dma_start(out=out[:, :, :].rearrange("n p c -> p n c"),
                              in_=o2_sb[:])

    nc.finalize()
    return nc


def kernel(**inputs):
    global LAST_RESULT
    inputs = {k: np.ascontiguousarray(np.asarray(v, np.float32)) for k, v in inputs.items()}
    if "nc" not in _CACHE:
        _CACHE["nc"] = _build()
    nc = _CACHE["nc"]
    c = _prep_consts(
        inputs["token_w"], inputs["patch_w"], inputs["patch_b"], inputs["Wi"],
        inputs["pconv_w"], inputs["pconv_b"], inputs["bn_g"], inputs["bn_b"],
        inputs["aconv_w"], inputs["fc1_w"], inputs["fc1_b"], inputs["fc2_w"],
        inputs["fc2_b"])
    xws, xq, xtp_full = _build_xw(inputs["x"])
    for q in range(3):
        c[f"xw{q}"] = xws[q]
    c["xq"] = xq
    import ml_dtypes
    in_maps = []
    for core in range(N_CORES):
        m = dict(c)
        xt = xtp_full[BPC * core:BPC * (core + 1)]        # (BPC, CIN, 194)
        x3 = np.zeros((128, BPC, L), np.float32)
        for k in range(3):
            x3[k * CIN:(k + 1) * CIN] = xt[:, :, k:k + L].transpose(1, 0, 2)
        x3[63:63 + 48] = c["peU"].T[:, None, :]
        m["xt3p"] = np.ascontiguousarray(x3).astype(ml_dtypes.bfloat16)
        in_maps.append(m)
    import os
    res = run_bass_kernel_spmd(nc, in_maps, core_ids=list(range(N_CORES)),
                               trace=bool(os.environ.get("BASS_TRACE")))
    LAST_RESULT = res
    return np.concatenate([res.results[cid]["out"] for cid in range(N_CORES)], axis=0)
